# revision 41
# baseline (speedup 1.0000x reference)
"""Trainium2 Bass kernel for nn_EntropyModel (MoE routing over K=4 class towers).

Strategy: every op in the tower is a per-pixel 1x1 conv (matmul over channels),
and the final one-hot masked sum selects exactly one class tower per pixel.
So route on the host: sort pixels by seg class, give each of the 8 cores a
slice of one class's pixel list (shard counts per class assigned
proportionally -- 2 cores per class when seg is balanced), run that class's
tower densely on its gathered pixels, and scatter the results back.

The 5-matmul tower is algebraically collapsed to 4 matmuls per pixel by
folding the linear layers around the two LeakyReLUs (host precomputes the
merged 128x128 weights):
    a2 = lrelu(V x + c)          V  = Wr1 W1,      c   = Wr1 b1 + br1
    h3 = lrelu(T x + U a2 + b3') T  = W3 W1,       U   = W3 Wr2,
                                 b3' = W3 (b1 + br2) + b3
    y  = W4 h3 + b4
Matmuls run in float32r (reduced-precision fp32 PE mode, ~1e-4 rel err per
matmul, 4x faster than full fp32).
"""
import numpy as np

import concourse.mybir as mybir
import concourse.tile as tile
from concourse import bacc
from concourse.bass_utils import run_bass_kernel_spmd

B, C, H, W = 2, 128, 192, 192
K = 4
O = 60
NTOT = B * H * W
NCORES = 8
MACRO = 1024  # free-dim per ACT/PSUM chunk (2 PSUM banks)
MMF = 512     # free-dim per matmul (1 PSUM bank, fp32)

F32 = mybir.dt.float32
F32R = mybir.dt.float32r

LAST_RESULTS = None  # test harness reads exec_time_ns off this

_nc_cache = {}


def _build(cap):
    nc = bacc.Bacc(None, target_bir_lowering=False)
    x = nc.dram_tensor("x", [C, cap], F32R, kind="ExternalInput")
    # packed weights [vt | tt], [ut | w4t]
    wpb = nc.dram_tensor("wpb", [C, 2 * C], F32R, kind="ExternalInput")
    wpr = nc.dram_tensor("wpr", [C, C + O], F32R, kind="ExternalInput")
    # packed biases: [c | b3' | b4(rows 0..59)]
    bp = nc.dram_tensor("bp", [C, 3], F32, kind="ExternalInput")
    y = nc.dram_tensor("y", [O, cap], F32, kind="ExternalOutput")

    # compute chunks: small first chunk to start the pipeline early; the
    # last chunk is the (ragged, 128-multiple) remainder
    spans = []
    s = 0
    while s < cap:
        rem = cap - s
        if s == 0 and cap > 2 * MACRO:
            w = MMF
        elif rem > MACRO:
            w = MACRO
        elif rem > MMF and cap > 2 * MACRO:
            w = rem - MMF  # leave a short 512-col final chunk
        else:
            w = rem
        spans.append((s, w))
        s += w

    Lrelu = mybir.ActivationFunctionType.Lrelu

    # Single integrated skew-2 pipeline over 1024-col chunks: the x stream,
    # PE, ACT, DVE and the y stream all overlap, and every dependency an
    # instruction waits on was produced >= 1 chunk earlier, so no engine's
    # in-order queue ever blocks ready work. 4 PSUM slots of 2 banks each.
    # Intermediates live full-size in SBUF (~12 MB of 26).
    with tile.TileContext(nc) as tc:
        with tc.tile_pool(name="const", bufs=1) as cw, \
             tc.tile_pool(name="big", bufs=1) as bigp, \
             tc.tile_pool(name="ps", bufs=4, space="PSUM") as ps:
            xt = bigp.tile([C, cap], F32R)
            a2t = bigp.tile([C, cap], F32R)
            h3t = bigp.tile([C, cap], F32R)
            yt = bigp.tile([O, cap], F32)

            # tiny bias DMA first primes the cold DMA queues, then weights
            # (needed by the first matmul), then x in slabs: two 1024-col
            # leading slabs so the first compute chunks unblock early, then
            # 2048-col slabs (decoupled from the compute chunking).
            bpt = cw.tile([C, 3], F32)
            nc.sync.dma_start(bpt[:], bp[:])
            wpbt = cw.tile([C, 2 * C], F32R)
            nc.sync.dma_start(wpbt[:], wpb[:])
            wprt = cw.tile([C, C + O], F32R)
            nc.sync.dma_start(wprt[:], wpr[:])
            s = 0
            for slab in (512, 1024):
                w = min(slab, cap - s)
                eng = nc.gpsimd if s == 0 else nc.sync
                eng.dma_start(xt[:, s:s + w], x[:, s:s + w])
                s += w
                if s >= cap:
                    break
            while s < cap:
                w = min(2048, cap - s)
                nc.sync.dma_start(xt[:, s:s + w], x[:, s:s + w])
                s += w

            vtt = wpbt[:, 0:C]
            ttt = wpbt[:, C:2 * C]
            utt = wprt[:, 0:C]
            w4tt = wprt[:, C:C + O]
            cbt = bpt[:, 0:1]
            b3t = bpt[:, 1:2]
            b4t = bpt[:O, 2:3]

            # PE warmup: HAM throttles the PE to 1.2 GHz until it has seen
            # ~3.4us of sustained matmul activity. Real work can't start
            # until the x stream delivers (~12us), so burn the wait on dummy
            # matmuls against a zeroed weight tile (no DMA dependency at all
            # -- rhs is garbage SBUF, results are discarded) to un-throttle
            # the clock before the first real matmul issues.
            wz = cw.tile([C, C], F32)
            nc.vector.memset(wz[:], 0.0)
            pwarm = ps.tile([C, MACRO], F32, tag="mm", name="pwarm")
            for _ in range(3):  # full-fp32 dummies: ~1.2us of PE busy each cold
                nc.tensor.matmul(pwarm[:, 0:MMF], wz[:],
                                 a2t[:, 0:MMF].bitcast(F32),
                                 start=True, stop=True)

            # skew-2 software pipeline: iteration c emits
            #   PE:  V(c), T(c), U(c-1), W4(c-2)   (deps are >= 1 iter old)
            #   ACT: a2act(c), h3act(c-1)
            #   DVE: bias-copy(c-2)
            # so neither PE's nor ACT's in-order queue ever blocks ready work.
            n_spans = len(spans)
            ph_tiles = {}
            ydone = 0
            for ci in range(n_spans + 2):
                if ci < n_spans:
                    s, w = spans[ci]
                    pa = ps.tile([C, MACRO], F32, tag="mm", name="pa")[:, :w]
                    for j in range(s, s + w, MMF):
                        n = min(MMF, s + w - j)
                        nc.tensor.matmul(pa[:, j - s:j - s + n], vtt,
                                         xt[:, j:j + n], start=True, stop=True)
                    nc.scalar.activation(a2t[:, s:s + w], pa[:], Lrelu,
                                         bias=cbt, scale=1.0, alpha=0.01)
                    ph = ps.tile([C, MACRO], F32, tag="mm", name="ph")[:, :w]
                    ph_tiles[ci] = ph
                    for j in range(s, s + w, MMF):
                        n = min(MMF, s + w - j)
                        nc.tensor.matmul(ph[:, j - s:j - s + n], ttt,
                                         xt[:, j:j + n], start=True, stop=False)
                if 0 <= ci - 1 < n_spans:
                    s, w = spans[ci - 1]
                    ph = ph_tiles.pop(ci - 1)
                    for j in range(s, s + w, MMF):
                        n = min(MMF, s + w - j)
                        nc.tensor.matmul(ph[:, j - s:j - s + n], utt,
                                         a2t[:, j:j + n], start=False, stop=True)
                    nc.scalar.activation(h3t[:, s:s + w], ph[:], Lrelu,
                                         bias=b3t, scale=1.0, alpha=0.01)
                if 0 <= ci - 2 < n_spans:
                    s, w = spans[ci - 2]
                    py = ps.tile([O, MACRO], F32, tag="mm", name="py")[:, :w]
                    for j in range(s, s + w, MMF):
                        n = min(MMF, s + w - j)
                        nc.tensor.matmul(py[:, j - s:j - s + n], w4tt,
                                         h3t[:, j:j + n], start=True, stop=True)
                    if ci - 2 == n_spans - 2:
                        # second-to-last copy on ACT so it overlaps the last
                        # chunk's copy on DVE during the wind-down
                        nc.scalar.activation(yt[:, s:s + w], py[:],
                                             mybir.ActivationFunctionType.Identity,
                                             bias=b4t, scale=1.0)
                    else:
                        nc.vector.tensor_scalar_add(yt[:, s:s + w], py[:], b4t)
                    thr = 1024 if ci - 2 >= n_spans - 3 else 2048
                    if s + w - ydone >= thr or ci - 2 == n_spans - 1:
                        nc.sync.dma_start(y[:, ydone:s + w], yt[:, ydone:s + w])
                        ydone = s + w
    nc.compile()
    return nc


def kernel(fusion_context, seg, W1, b1, Wr1, br1, Wr2, br2, W3, b3, W4, b4):
    global LAST_RESULTS
    fusion_context = np.asarray(fusion_context, dtype=np.float32)
    seg = np.asarray(seg)

    # [B,C,H,W] -> [C, B*H*W]; column n = (b, h, w) row-major
    xcols = np.ascontiguousarray(
        fusion_context.transpose(1, 0, 2, 3).reshape(C, NTOT))
    segf = seg.reshape(-1).astype(np.int64)

    # Route: give each core a slice of one class's pixel list. Shard counts
    # per class are assigned greedily (largest n_k/m_k gets the next shard)
    # so any seg distribution stays balanced and the per-core capacity is
    # bounded by ~NTOT/8.
    cls_ix = [np.nonzero(segf == k)[0] for k in range(K)]
    m = [1 if len(ix) > 0 else 0 for ix in cls_ix]
    if sum(m) == 0:
        m[0] = 1  # degenerate: no pixels at all; keep one dummy shard class
    while sum(m) < NCORES:
        k = max(range(K), key=lambda kk: len(cls_ix[kk]) / m[kk] if m[kk] else -1)
        m[k] += 1
    shards = []  # (class_id, column_indices)
    for k in range(K):
        parts = np.array_split(cls_ix[k], m[k]) if m[k] else []
        shards.extend((k, p) for p in parts)
    assert len(shards) == NCORES

    # SBUF holds ~12.5k columns of x/a2/h3/y comfortably; in the pathological
    # case of extreme class imbalance (cap up to ~NTOT/5), split every shard
    # in half and run the device kernel twice.
    cap = max(len(ix) for _, ix in shards)
    runs = [shards]
    if cap > 12288:
        runs = [[(k, ix[:(len(ix) + 1) // 2]) for k, ix in shards],
                [(k, ix[(len(ix) + 1) // 2:]) for k, ix in shards]]
        cap = max(len(ix) for r in runs for _, ix in r)
    cap = max(MMF, -(-cap // 128) * 128)  # round up to 128 columns

    if cap not in _nc_cache:
        _nc_cache[cap] = _build(cap)
    nc = _nc_cache[cap]

    f64 = np.float64

    def build_in_map(k, ix):
        xs = np.zeros((C, cap), dtype=np.float32)
        xs[:, :len(ix)] = xcols[:, ix]
        V = W1[k].astype(f64).T @ Wr1[k].astype(f64).T    # (Wr1 W1)^T
        T = W1[k].astype(f64).T @ W3[k].astype(f64).T     # (W3 W1)^T
        U = Wr2[k].astype(f64).T @ W3[k].astype(f64).T    # (W3 Wr2)^T
        c = Wr1[k].astype(f64) @ b1[k].astype(f64) + br1[k].astype(f64)
        b3p = W3[k].astype(f64) @ (b1[k].astype(f64) + br2[k].astype(f64)) \
            + b3[k].astype(f64)
        wpb = np.concatenate([V, T], axis=1).astype(np.float32)
        wpr = np.concatenate(
            [U, W4[k].T.astype(f64)], axis=1).astype(np.float32)
        bp = np.zeros((C, 3), dtype=np.float32)
        bp[:, 0] = c
        bp[:, 1] = b3p
        bp[:O, 2] = b4[k]
        return {
            "x": xs,
            "wpb": np.ascontiguousarray(wpb),
            "wpr": np.ascontiguousarray(wpr),
            "bp": bp,
        }

    out = np.empty((O, NTOT), dtype=np.float32)
    for run_shards in runs:
        in_maps = [build_in_map(k, ix) for k, ix in run_shards]
        res = run_bass_kernel_spmd(nc, in_maps, core_ids=list(range(NCORES)))
        LAST_RESULTS = res
        for (k, ix), r in zip(run_shards, res.results):
            out[:, ix] = r["y"][:, :len(ix)]
    return np.ascontiguousarray(
        out.reshape(O, B, H * W).transpose(1, 0, 2).reshape(B, O, H, W))


# revision 42
# speedup vs baseline: 1.0128x; 1.0128x over previous
"""Trainium2 Bass kernel for nn_EntropyModel (MoE routing over K=4 class towers).

Strategy: every op in the tower is a per-pixel 1x1 conv (matmul over channels),
and the final one-hot masked sum selects exactly one class tower per pixel.
So route on the host: sort pixels by seg class, give each of the 8 cores a
slice of one class's pixel list (shard counts per class assigned
proportionally -- 2 cores per class when seg is balanced), run that class's
tower densely on its gathered pixels, and scatter the results back.

The 5-matmul tower is algebraically collapsed to 4 matmuls per pixel by
folding the linear layers around the two LeakyReLUs (host precomputes the
merged 128x128 weights):
    a2 = lrelu(V x + c)          V  = Wr1 W1,      c   = Wr1 b1 + br1
    h3 = lrelu(T x + U a2 + b3') T  = W3 W1,       U   = W3 Wr2,
                                 b3' = W3 (b1 + br2) + b3
    y  = W4 h3 + b4
Matmuls run in float32r (reduced-precision fp32 PE mode, ~1e-4 rel err per
matmul, 4x faster than full fp32).
"""
import numpy as np

import concourse.mybir as mybir
import concourse.tile as tile
from concourse import bacc
from concourse.bass_utils import run_bass_kernel_spmd

B, C, H, W = 2, 128, 192, 192
K = 4
O = 60
NTOT = B * H * W
NCORES = 8
MACRO = 1024  # free-dim per ACT/PSUM chunk (2 PSUM banks)
MMF = 512     # free-dim per matmul (1 PSUM bank, fp32)

F32 = mybir.dt.float32
F32R = mybir.dt.float32r

LAST_RESULTS = None  # test harness reads exec_time_ns off this

_nc_cache = {}


def _build(cap):
    nc = bacc.Bacc(None, target_bir_lowering=False)
    x = nc.dram_tensor("x", [C, cap], F32R, kind="ExternalInput")
    # packed weights [vt | tt], [ut | w4t]
    wpb = nc.dram_tensor("wpb", [C, 2 * C], F32R, kind="ExternalInput")
    wpr = nc.dram_tensor("wpr", [C, C + O], F32R, kind="ExternalInput")
    # packed biases: [c | b3' | b4(rows 0..59)]
    bp = nc.dram_tensor("bp", [C, 3], F32, kind="ExternalInput")
    y = nc.dram_tensor("y", [O, cap], F32, kind="ExternalOutput")

    # compute chunks: small first chunk to start the pipeline early; the
    # last chunk is the (ragged, 128-multiple) remainder
    spans = []
    s = 0
    while s < cap:
        rem = cap - s
        if s == 0 and cap > 2 * MACRO:
            w = MMF
        else:
            w = min(MACRO, rem)
        spans.append((s, w))
        s += w

    Lrelu = mybir.ActivationFunctionType.Lrelu

    # Single integrated skew-2 pipeline over 1024-col chunks: the x stream,
    # PE, ACT, DVE and the y stream all overlap, and every dependency an
    # instruction waits on was produced >= 1 chunk earlier, so no engine's
    # in-order queue ever blocks ready work. 4 PSUM slots of 2 banks each.
    # Intermediates live full-size in SBUF (~12 MB of 26).
    with tile.TileContext(nc) as tc:
        with tc.tile_pool(name="const", bufs=1) as cw, \
             tc.tile_pool(name="big", bufs=1) as bigp, \
             tc.tile_pool(name="ps", bufs=4, space="PSUM") as ps:
            xt = bigp.tile([C, cap], F32R)
            a2t = bigp.tile([C, cap], F32R)
            h3t = bigp.tile([C, cap], F32R)
            yt = bigp.tile([O, cap], F32)

            # tiny bias DMA first primes the cold DMA queues, then weights
            # (needed by the first matmul), then x in slabs: two 1024-col
            # leading slabs so the first compute chunks unblock early, then
            # 2048-col slabs (decoupled from the compute chunking).
            bpt = cw.tile([C, 3], F32)
            nc.sync.dma_start(bpt[:], bp[:])
            wpbt = cw.tile([C, 2 * C], F32R)
            nc.sync.dma_start(wpbt[:], wpb[:])
            wprt = cw.tile([C, C + O], F32R)
            nc.sync.dma_start(wprt[:], wpr[:])
            s = 0
            for slab in (512, 1024):
                w = min(slab, cap - s)
                nc.sync.dma_start(xt[:, s:s + w], x[:, s:s + w])
                s += w
                if s >= cap:
                    break
            while s < cap:
                w = min(2048, cap - s)
                nc.sync.dma_start(xt[:, s:s + w], x[:, s:s + w])
                s += w

            vtt = wpbt[:, 0:C]
            ttt = wpbt[:, C:2 * C]
            utt = wprt[:, 0:C]
            w4tt = wprt[:, C:C + O]
            cbt = bpt[:, 0:1]
            b3t = bpt[:, 1:2]
            b4t = bpt[:O, 2:3]

            # PE warmup: HAM throttles the PE to 1.2 GHz until it has seen
            # ~3.4us of sustained matmul activity. Real work can't start
            # until the x stream delivers (~12us), so burn the wait on dummy
            # matmuls against a zeroed weight tile (no DMA dependency at all
            # -- rhs is garbage SBUF, results are discarded) to un-throttle
            # the clock before the first real matmul issues.
            wz = cw.tile([C, C], F32)
            nc.vector.memset(wz[:], 0.0)
            pwarm = ps.tile([C, MACRO], F32, tag="mm", name="pwarm")
            for _ in range(3):  # full-fp32 dummies: ~1.2us of PE busy each cold
                nc.tensor.matmul(pwarm[:, 0:MMF], wz[:],
                                 a2t[:, 0:MMF].bitcast(F32),
                                 start=True, stop=True)

            # skew-2 software pipeline: iteration c emits
            #   PE:  V(c), T(c), U(c-1), W4(c-2)   (deps are >= 1 iter old)
            #   ACT: a2act(c), h3act(c-1)
            #   DVE: bias-copy(c-2)
            # so neither PE's nor ACT's in-order queue ever blocks ready work.
            n_spans = len(spans)
            ph_tiles = {}
            ydone = 0
            for ci in range(n_spans + 2):
                if ci < n_spans:
                    s, w = spans[ci]
                    pa = ps.tile([C, MACRO], F32, tag="mm", name="pa")[:, :w]
                    for j in range(s, s + w, MMF):
                        n = min(MMF, s + w - j)
                        nc.tensor.matmul(pa[:, j - s:j - s + n], vtt,
                                         xt[:, j:j + n], start=True, stop=True)
                    nc.scalar.activation(a2t[:, s:s + w], pa[:], Lrelu,
                                         bias=cbt, scale=1.0, alpha=0.01)
                    ph = ps.tile([C, MACRO], F32, tag="mm", name="ph")[:, :w]
                    ph_tiles[ci] = ph
                    for j in range(s, s + w, MMF):
                        n = min(MMF, s + w - j)
                        nc.tensor.matmul(ph[:, j - s:j - s + n], ttt,
                                         xt[:, j:j + n], start=True, stop=False)
                if 0 <= ci - 1 < n_spans:
                    s, w = spans[ci - 1]
                    ph = ph_tiles.pop(ci - 1)
                    for j in range(s, s + w, MMF):
                        n = min(MMF, s + w - j)
                        nc.tensor.matmul(ph[:, j - s:j - s + n], utt,
                                         a2t[:, j:j + n], start=False, stop=True)
                    nc.scalar.activation(h3t[:, s:s + w], ph[:], Lrelu,
                                         bias=b3t, scale=1.0, alpha=0.01)
                if 0 <= ci - 2 < n_spans:
                    s, w = spans[ci - 2]
                    py = ps.tile([O, MACRO], F32, tag="mm", name="py")[:, :w]
                    for j in range(s, s + w, MMF):
                        n = min(MMF, s + w - j)
                        nc.tensor.matmul(py[:, j - s:j - s + n], w4tt,
                                         h3t[:, j:j + n], start=True, stop=True)
                    if ci - 2 == n_spans - 2:
                        # second-to-last copy on ACT so it overlaps the last
                        # chunk's copy on DVE during the wind-down
                        nc.scalar.activation(yt[:, s:s + w], py[:],
                                             mybir.ActivationFunctionType.Identity,
                                             bias=b4t, scale=1.0)
                    else:
                        nc.vector.tensor_scalar_add(yt[:, s:s + w], py[:], b4t)
                    thr = 1024 if ci - 2 >= n_spans - 3 else 2048
                    if s + w - ydone >= thr or ci - 2 == n_spans - 1:
                        nc.sync.dma_start(y[:, ydone:s + w], yt[:, ydone:s + w])
                        ydone = s + w
    nc.compile()
    return nc


def kernel(fusion_context, seg, W1, b1, Wr1, br1, Wr2, br2, W3, b3, W4, b4):
    global LAST_RESULTS
    fusion_context = np.asarray(fusion_context, dtype=np.float32)
    seg = np.asarray(seg)

    # [B,C,H,W] -> [C, B*H*W]; column n = (b, h, w) row-major
    xcols = np.ascontiguousarray(
        fusion_context.transpose(1, 0, 2, 3).reshape(C, NTOT))
    segf = seg.reshape(-1).astype(np.int64)

    # Route: give each core a slice of one class's pixel list. Shard counts
    # per class are assigned greedily (largest n_k/m_k gets the next shard)
    # so any seg distribution stays balanced and the per-core capacity is
    # bounded by ~NTOT/8.
    cls_ix = [np.nonzero(segf == k)[0] for k in range(K)]
    m = [1 if len(ix) > 0 else 0 for ix in cls_ix]
    if sum(m) == 0:
        m[0] = 1  # degenerate: no pixels at all; keep one dummy shard class
    while sum(m) < NCORES:
        k = max(range(K), key=lambda kk: len(cls_ix[kk]) / m[kk] if m[kk] else -1)
        m[k] += 1
    shards = []  # (class_id, column_indices)
    for k in range(K):
        parts = np.array_split(cls_ix[k], m[k]) if m[k] else []
        shards.extend((k, p) for p in parts)
    assert len(shards) == NCORES

    # SBUF holds ~12.5k columns of x/a2/h3/y comfortably; in the pathological
    # case of extreme class imbalance (cap up to ~NTOT/5), split every shard
    # in half and run the device kernel twice.
    cap = max(len(ix) for _, ix in shards)
    runs = [shards]
    if cap > 12288:
        runs = [[(k, ix[:(len(ix) + 1) // 2]) for k, ix in shards],
                [(k, ix[(len(ix) + 1) // 2:]) for k, ix in shards]]
        cap = max(len(ix) for r in runs for _, ix in r)
    cap = max(MMF, -(-cap // 128) * 128)  # round up to 128 columns

    if cap not in _nc_cache:
        _nc_cache[cap] = _build(cap)
    nc = _nc_cache[cap]

    f64 = np.float64

    def build_in_map(k, ix):
        xs = np.zeros((C, cap), dtype=np.float32)
        xs[:, :len(ix)] = xcols[:, ix]
        V = W1[k].astype(f64).T @ Wr1[k].astype(f64).T    # (Wr1 W1)^T
        T = W1[k].astype(f64).T @ W3[k].astype(f64).T     # (W3 W1)^T
        U = Wr2[k].astype(f64).T @ W3[k].astype(f64).T    # (W3 Wr2)^T
        c = Wr1[k].astype(f64) @ b1[k].astype(f64) + br1[k].astype(f64)
        b3p = W3[k].astype(f64) @ (b1[k].astype(f64) + br2[k].astype(f64)) \
            + b3[k].astype(f64)
        wpb = np.concatenate([V, T], axis=1).astype(np.float32)
        wpr = np.concatenate(
            [U, W4[k].T.astype(f64)], axis=1).astype(np.float32)
        bp = np.zeros((C, 3), dtype=np.float32)
        bp[:, 0] = c
        bp[:, 1] = b3p
        bp[:O, 2] = b4[k]
        return {
            "x": xs,
            "wpb": np.ascontiguousarray(wpb),
            "wpr": np.ascontiguousarray(wpr),
            "bp": bp,
        }

    out = np.empty((O, NTOT), dtype=np.float32)
    for run_shards in runs:
        in_maps = [build_in_map(k, ix) for k, ix in run_shards]
        res = run_bass_kernel_spmd(nc, in_maps, core_ids=list(range(NCORES)))
        LAST_RESULTS = res
        for (k, ix), r in zip(run_shards, res.results):
            out[:, ix] = r["y"][:, :len(ix)]
    return np.ascontiguousarray(
        out.reshape(O, B, H * W).transpose(1, 0, 2).reshape(B, O, H, W))
